# revision 18
# baseline (speedup 1.0000x reference)
"""Trainium2 Bass kernel: additive (Bahdanau) cross attention.

  att_en = en_seq @ w_en                      (B, T_en, U)
  att_de = de_seq @ w_de                      (B, T_de, U)
  mu[b,t,e] = sum_u tanh(att_en[b,e,u] + att_de[b,t,u]) * nu[u]
  alphas = softmax(mu, axis=e)
  out = de_seq + alphas @ en_seq

Sharding: data-parallel over batch, one batch element per NeuronCore
(B == 8 == n_cores), weights replicated.  No collectives.

Per-core dataflow:
  - PE: att_enT[u,e], att_deT[u,t] projections (lhsT = w native layout, f32)
  - DVE tensor_scalar_add (2x mode): co[u, j, e] = att_enT[u,e] + att_deT[u,t]
  - ACT: tanh over [128, KT*256] staging tiles, f32 in -> bf16 out (the
    16.7M-element bottleneck; 1 elem/lane/cycle at 1.2 GHz)
  - PE matvec trick (bf16): lhsT is a 128-wide sliding window into a
    zeros-padded buffer holding nu at column 128, so matmul t writes
    nu.T @ tanh_co to PSUM partition (t mod 128) and adds zeros everywhere
    else.  512 N=256 matmuls accumulate a [128, 256] mu block per 128
    decoder steps.
  - softmax over e on [128, 256] (reduce_max(negate) -> Exp(bias=-max) ->
    reduce_sum -> reciprocal -> tensor_scalar_mul); tanh and exp share the
    ACT table set so there is a single table load.
  - PE transpose of alphas + 2 matmuls against en_seq chunks, DVE adds
    de_seq residual, DMA out.
"""

import numpy as np

B, T_EN, T_DE, D, U = 8, 256, 256, 256, 256
P = 128
N_CORES = 8
KT = 32  # decoder steps per ACT staging group

_CACHE = {}


def _build(loop_n=None):
    """Build the kernel graph. loop_n: if set, wrap the compute body in a
    For_i that repeats it loop_n times (for HW timing via slope)."""
    import concourse.bacc as bacc
    import concourse.mybir as mybir
    from concourse.tile import TileContext
    from concourse.masks import make_identity

    f32 = mybir.dt.float32
    bf16 = mybir.dt.bfloat16
    Tanh = mybir.ActivationFunctionType.Tanh
    Exp = mybir.ActivationFunctionType.Exp
    AX = mybir.AxisListType.X

    nc = bacc.Bacc("TRN2", target_bir_lowering=False, debug=False)

    # packp[p, c, :]: w_en | w_de | enT | deT rows (c*128+p) in bf16
    # packe[p, c, :]: en rows in bf16
    # packf[p, c, :]: de row | nu value | pad, in f32
    packp = nc.dram_tensor("packp", [P, 2, 4 * 256], bf16, kind="ExternalInput")
    packe = nc.dram_tensor("packe", [P, 2, 256], bf16, kind="ExternalInput")
    packf = nc.dram_tensor("packf", [P, 2, 258], f32, kind="ExternalInput")
    out = nc.dram_tensor("out", [T_DE, D], f32, kind="ExternalOutput")  # [t, d]

    with TileContext(nc) as tc:
        with (
            tc.tile_pool(name="consts", bufs=1) as consts,
            tc.tile_pool(name="cop", bufs=4) as cop,
            tc.tile_pool(name="smax", bufs=2) as smax,
            tc.tile_pool(name="small", bufs=4) as small,
            tc.tile_pool(name="mu_pp", bufs=2, space="PSUM") as mu_pp,
            tc.tile_pool(name="tr_pp", bufs=2, space="PSUM") as tr_pp,
            tc.tile_pool(name="acc_pp", bufs=2, space="PSUM") as acc_pp,
        ):
            # ---------------- constants / input staging ----------------
            ident = consts.tile([P, P], bf16)
            make_identity(nc, ident)

            packp_sb = consts.tile([P, 2, 4 * 256], bf16)
            packe_sb = consts.tile([P, 2, 256], bf16)
            packf_sb = consts.tile([P, 2, 258], f32)
            # views into the packed staging tiles
            w_en_sb = packp_sb[:, :, 0:256]  # [d%128, d//128, u]
            w_de_sb = packp_sb[:, :, 256:512]
            enT_sb = packp_sb[:, :, 512:768]  # [d%128, d//128, e]
            deT_sb = packp_sb[:, :, 768:1024]  # [d%128, d//128, t]
            en_sb = packe_sb[:, :, :]  # [e%128, e//128, d]
            de_sb = packf_sb[:, :, 0:256]  # [t%128, t//128, d]
            nusb = packf_sb[:, :, 256:257]  # [p, c, 1]
            # zeros with nu_chunk at column P: sliding lhsT window puts
            # nu at output partition t%128 of the matvec matmul.
            nuz = consts.tile([P, 2, 2 * P], bf16)

            nc.sync.dma_start(out=packp_sb[:], in_=packp[:, :, :])
            nc.scalar.dma_start(out=packe_sb[:], in_=packe[:, :, :])
            nc.gpsimd.dma_start(out=packf_sb[:], in_=packf[:, :, :])

            nc.gpsimd.memset(nuz[:], 0.0)
            for c in range(2):
                nc.vector.tensor_copy(out=nuz[:, c, P:P + 1], in_=nusb[:, c, :])

            att_enT = consts.tile([P, 2, T_EN], bf16)  # [u%128, u//128, e]
            att_deT = consts.tile([P, 2, T_DE], f32)  # [u%128, u//128, t]

            def emit_body():
                # ---------------- projections ----------------
                # att_enT[u, e] = sum_d w_en[d, u] * enT[d, e]  (u in 2 chunks)
                for cu in range(2):
                    for xT_sb, w_sb, attT in (
                        (enT_sb, w_en_sb, att_enT),
                        (deT_sb, w_de_sb, att_deT),
                    ):
                        pp = acc_pp.tile([P, 256], f32, tag="pp", name="pp")
                        for cd in range(2):
                            nc.tensor.matmul(
                                out=pp[:],
                                lhsT=w_sb[:, cd, cu * P:(cu + 1) * P],
                                rhs=xT_sb[:, cd, :],
                                start=(cd == 0),
                                stop=(cd == 1),
                            )
                        nc.vector.tensor_copy(out=attT[:, cu, :], in_=pp[:])

                # ---------------- main loop ----------------
                n_blk = T_DE // P
                for blk in range(n_blk):
                    # taper the first block's head (ACT starts after only
                    # 4 DVE adds) and the last block's tail (short post-tanh
                    # critical chain: matvec + softmax + epilogue)
                    if blk == 0:
                        sizes = [4, 4, 8, 16, 32, 32, 32]
                    elif blk == n_blk - 1:
                        sizes = [32, 32, 32, 16, 8, 4, 4]
                    else:
                        sizes = [KT] * (P // KT)
                    assert sum(sizes) == P
                    mu_ps = mu_pp.tile([P, T_EN], f32, tag="mu", name="mu_ps")
                    tm_base = 0
                    for g, gsz in enumerate(sizes):
                        ths = []
                        for c in range(2):
                            co = cop.tile(
                                [P, gsz, T_EN], bf16, tag=f"co{c}", name="co"
                            )
                            for j in range(gsz):
                                t = blk * P + tm_base + j
                                nc.vector.tensor_scalar_add(
                                    out=co[:, j, :],
                                    in0=att_enT[:, c, :],
                                    scalar1=att_deT[:, c, t:t + 1],
                                )
                            nc.scalar.activation(out=co[:], in_=co[:], func=Tanh)
                            ths.append(co)
                        for j in range(gsz):
                            tm = tm_base + j  # t mod 128
                            for c in range(2):
                                nc.tensor.matmul(
                                    out=mu_ps[:],
                                    lhsT=nuz[:, c, P - tm:2 * P - tm],
                                    rhs=ths[c][:, j, :],
                                    start=(tm == 0 and c == 0),
                                    stop=(tm == P - 1 and c == 1),
                                )
                        tm_base += gsz

                    # softmax over e, unnormalized: expm[t, e], row sums
                    # fused into the Exp via accum_out; 1/sum applied at the
                    # end (after the en-matmul) so the transposes start early.
                    mx = small.tile([P, 1], f32, tag="mx", name="mx")
                    nc.vector.reduce_max(
                        out=mx[:], in_=mu_ps[:], axis=AX, negate=True
                    )
                    expm = smax.tile([P, T_EN], bf16, tag="expm", name="expm")
                    sm = small.tile([P, 1], f32, tag="sm", name="sm")
                    nc.scalar.activation(
                        out=expm[:], in_=mu_ps[:], func=Exp, bias=mx[:, 0:1],
                        scale=1.0, accum_out=sm[:],
                    )
                    rc = small.tile([P, 1], f32, tag="rc", name="rc")
                    nc.vector.reciprocal(out=rc[:], in_=sm[:])

                    # unnorm[t, d] = sum_e expm[t, e] * en[e, d]
                    aT = smax.tile([P, 2, P], bf16, tag="aT", name="aT")
                    for c in range(2):
                        trp = tr_pp.tile([P, P], bf16, tag="trp", name="trp")
                        nc.tensor.transpose(
                            out=trp[:],
                            in_=expm[:, c * P:(c + 1) * P],
                            identity=ident[:],
                        )
                        nc.vector.tensor_copy(out=aT[:, c, :], in_=trp[:])
                    acc = acc_pp.tile([P, D], f32, tag="pp", name="acc")
                    for c in range(2):
                        nc.tensor.matmul(
                            out=acc[:],
                            lhsT=aT[:, c, :],
                            rhs=en_sb[:, c, :],
                            start=(c == 0),
                            stop=(c == 1),
                        )
                    # ob = acc * rc + de  (both on DVE; ACT stays on tanh)
                    ob = smax.tile([P, D], f32, tag="ob", name="ob")
                    nc.vector.tensor_scalar_mul(
                        out=ob[:], in0=acc[:], scalar1=rc[:, 0:1]
                    )
                    nc.vector.tensor_add(out=ob[:], in0=ob[:], in1=de_sb[:, blk, :])
                    nc.gpsimd.dma_start(out=out[blk * P:(blk + 1) * P, :], in_=ob[:])

            if loop_n is None:
                emit_body()
            else:
                hint = (
                    mybir.EngineType.PE,
                    mybir.EngineType.DVE,
                    mybir.EngineType.Activation,
                )
                with tc.For_i(0, loop_n, 1, hint_engines=hint):
                    emit_body()

    nc.compile()
    return nc


def _get_nc(loop_n=None):
    key = ("nc", loop_n)
    if key not in _CACHE:
        _CACHE[key] = _build(loop_n)
    return _CACHE[key]


def make_in_maps(inputs):
    import ml_dtypes

    bf = ml_dtypes.bfloat16
    en_seq = np.asarray(inputs["en_seq"], dtype=np.float32)
    de_seq = np.asarray(inputs["de_seq"], dtype=np.float32)
    w_en = np.asarray(inputs["w_en"], dtype=np.float32)
    w_de = np.asarray(inputs["w_de"], dtype=np.float32)
    nu = np.asarray(inputs["nu"], dtype=np.float32)

    enT = en_seq.transpose(0, 2, 1)  # [B, d, e]
    deT = de_seq.transpose(0, 2, 1)  # [B, d, t]

    in_maps = []
    for b in range(B):
        # packp[p, c, :] = w_en|w_de|enT|deT rows (c*128+p), bf16
        packp = np.empty((P, 2, 4 * 256), dtype=bf)
        packe = np.empty((P, 2, 256), dtype=bf)
        packf = np.zeros((P, 2, 258), dtype=np.float32)
        for c in range(2):
            rows = slice(c * P, (c + 1) * P)
            packp[:, c, 0:256] = w_en[rows, :].astype(bf)
            packp[:, c, 256:512] = w_de[rows, :].astype(bf)
            packp[:, c, 512:768] = enT[b][rows, :].astype(bf)
            packp[:, c, 768:1024] = deT[b][rows, :].astype(bf)
            packe[:, c, :] = en_seq[b][rows, :].astype(bf)
            packf[:, c, 0:256] = de_seq[b][rows, :]
            packf[:, c, 256] = nu[rows, 0]
        in_maps.append(
            {"packp": np.ascontiguousarray(packp),
             "packe": np.ascontiguousarray(packe),
             "packf": np.ascontiguousarray(packf)}
        )
    return in_maps


def kernel(**inputs):
    from concourse.bass_utils import run_bass_kernel_spmd

    in_maps = make_in_maps(inputs)
    nc = _get_nc()
    res = run_bass_kernel_spmd(nc, in_maps, core_ids=list(range(N_CORES)))
    return np.stack([res.results[b]["out"] for b in range(B)], axis=0)


if __name__ == "__main__":
    rng = np.random.default_rng(0)
    ins = {
        "en_seq": rng.standard_normal((B, T_EN, D), dtype=np.float32),
        "de_seq": rng.standard_normal((B, T_DE, D), dtype=np.float32),
        "w_en": rng.standard_normal((D, U), dtype=np.float32) / np.sqrt(D),
        "w_de": rng.standard_normal((D, U), dtype=np.float32) / np.sqrt(D),
        "nu": rng.standard_normal((U, 1), dtype=np.float32) / np.sqrt(U),
    }
    out = kernel(**ins)
    print(out.shape, out.dtype)
